# revision 13
# baseline (speedup 1.0000x reference)
"""Dilated attention (segment 64, dilation 4, 16 heads, head_dim 64) on 8 trn2 cores.

Sharding: 2 batches x 4 head-groups (4 heads each) = 8 cores. Each core computes
q/k/v projections for its 4 heads on its batch, block-sparse attention over the
+-2 block (256-token) dilated band, and a partial output projection. Host sums
the 4 head-group partials per batch.

Layout is fully "transposed" on-core to avoid PE transposes:
  xT   [D, S]    (D on partitions, 8 chunks of 128)
  qT/kT [64h, S] per head (head dim on partitions)
  v    [S, 64]   natural (keys on partitions) + ones column -> softmax denoms
  scoresT [k-block 128, q-window <=640] = kT_blk-stationary x qT-window

v3 changes vs v2 (151949 -> 146576 measured):
  - pre-era proj psum double-buffered (bufs=2) so q/k copy overlaps the
    next accumulation chain.
  - pair-1 q/k projections run dc-outer (weights stationary across 4
    query-tile psums) to amortize LDWEIGHTS.
  - exp+mask emitted per 512/128 piece (finer PV wake-up).
  - PV is quarter-major: each (pair,head) accumulates one [65,512]
    quarter psum; its full-cover piece (kb=4qt+2) is emitted first with
    start=True so the whole-bank zero covers every column.
  - normalize: den copy on DVE (NOT ACT -- a PV-dependent copy in the
    exp-heavy ACT FIFO stalls the at-chain and depresses the PE duty
    cycle enough to keep the HAM clock throttled ~20%), and NOT
    high-priority (late scheduling keeps it from blocking mask-muls).
    reciprocal_approx_fast must read SBUF: reading PSUM directly
    returns garbage on HW (sim accepts it).
  - y projection + output DMA per query-quarter, interleaved with
    scores_p1 groups and v-proj chunks as PE gap fillers; tail is only
    the last quarter's yproj + 1MB DMA.
"""

import numpy as np
import ml_dtypes

bfloat16 = ml_dtypes.bfloat16

B, S, D = 2, 2048, 1024
H, Dh = 16, 64
NCORES = 8
NKB = S // 128  # 16 key blocks
WMAX = 640

_cache = {}


def _mask_rel():
    kp = np.arange(128)[:, None]
    j = np.arange(WMAX)[None, :]
    qrel = j - 256
    diff = np.abs(qrel - kp)
    seg = (qrel // 64) == (kp // 64)
    dil = (diff > 0) & (diff % 4 == 0) & (diff <= 256)
    m = (seg | dil).astype(bfloat16)
    return np.ascontiguousarray(np.stack([m, m], axis=1))  # [128, 2, WMAX]


def _win(kb):
    return max(0, kb - 2) * 128, min(NKB, kb + 3) * 128


def _build(debug=False):
    key = ("nc", debug)
    if key in _cache:
        return _cache[key]
    import concourse.mybir as mybir
    from concourse import bacc
    from concourse.tile import TileContext

    bf = mybir.dt.bfloat16
    f32 = mybir.dt.float32
    EXP = mybir.ActivationFunctionType.Exp

    nc = bacc.Bacc()
    d_x = nc.declare_dram_parameter("xT", [128, 8, S], bf, isOutput=False)
    d_wq0 = nc.declare_dram_parameter("wq0", [128, 8, 128], bf, isOutput=False)
    d_wq1 = nc.declare_dram_parameter("wq1", [128, 8, 128], bf, isOutput=False)
    d_wk0 = nc.declare_dram_parameter("wk0", [128, 8, 128], bf, isOutput=False)
    d_wk1 = nc.declare_dram_parameter("wk1", [128, 8, 128], bf, isOutput=False)
    d_wv = nc.declare_dram_parameter("wv", [128, 8, 256], bf, isOutput=False)
    d_wo = nc.declare_dram_parameter("wo", [128, 2, 1024], bf, isOutput=False)
    d_mask = nc.declare_dram_parameter("maskT", [128, 2, WMAX], bf, isOutput=False)
    d_y = nc.declare_dram_parameter("yT", [128, 8, S], bf, isOutput=True)
    if debug:
        d_dbg_q = nc.declare_dram_parameter("dbg_q", [128, S], bf, isOutput=True)
        d_dbg_k = nc.declare_dram_parameter("dbg_k", [128, S], bf, isOutput=True)
        d_dbg_v = nc.declare_dram_parameter("dbg_v", [128, 16 * 4 * 65], bf, isOutput=True)
        d_dbg_at = nc.declare_dram_parameter("dbg_at", [128, WMAX], bf, isOutput=True)
        d_dbg_on = nc.declare_dram_parameter("dbg_on", [128, S], bf, isOutput=True)

    with TileContext(nc) as tc:
        with (
            tc.tile_pool(name="const", bufs=1) as cpool,
            tc.tile_pool(name="attn", bufs=30) as apool,
            tc.tile_pool(name="ysb", bufs=6) as ypool,
            tc.tile_pool(name="small", bufs=4) as spool,
        ):
            sb_wq = cpool.tile([128, 8, 2, 128], bf, name="wq", tag="wq")
            sb_wk = cpool.tile([128, 8, 2, 128], bf, name="wk", tag="wk")
            sb_wv = cpool.tile([128, 8, 256], bf, name="wv", tag="wv")
            sb_wo = cpool.tile([128, 2, 1024], bf, name="wo", tag="wo")
            sb_mask = cpool.tile([128, 2, WMAX], bf, name="mask", tag="mask")
            sb_xall = cpool.tile([128, 8, S], bf, name="xall", tag="xall")
            sb_x = [sb_xall[:, dc, :] for dc in range(8)]

            junk = cpool.tile([128, 256], bf, name="junk", tag="junk")
            nc.gpsimd.memset(junk, 0.0)

            # ---- input DMA priming: pair-0 weights + x-tt0 first on all
            # 3 rings; later-needed tensors (mask/wv/wq1/wk1/wo) after the
            # x chunks they compete with.
            nc.scalar.dma_start(out=sb_wq[:, :, 0, :], in_=d_wq0[:, :, :])
            nc.sync.dma_start(out=sb_wk[:, :, 0, :], in_=d_wk0[:, :, :])
            nc.gpsimd.dma_start(out=sb_mask, in_=d_mask[:, :, :])
            for tt in range(4):
                sl = slice(tt * 512, (tt + 1) * 512)
                nc.scalar.dma_start(out=sb_xall[:, 0:3, sl], in_=d_x[:, 0:3, sl])
                nc.sync.dma_start(out=sb_xall[:, 3:6, sl], in_=d_x[:, 3:6, sl])
                nc.gpsimd.dma_start(out=sb_xall[:, 6:8, sl], in_=d_x[:, 6:8, sl])
            nc.scalar.dma_start(out=sb_wq[:, :, 1, :], in_=d_wq1[:, :, :])
            nc.sync.dma_start(out=sb_wk[:, :, 1, :], in_=d_wk1[:, :, :])
            nc.gpsimd.dma_start(out=sb_wv[:, :, :], in_=d_wv[:, :, :])
            nc.sync.dma_start(out=sb_wo[:, 0, :], in_=d_wo[:, 0, :])
            nc.gpsimd.dma_start(out=sb_wo[:, 1, :], in_=d_wo[:, 1, :])

            sb_q = []
            sb_k = []
            sb_on = []
            for p in range(2):
                sb_q.append(cpool.tile([128, S], bf, name=f"q{p}", tag=f"q{p}"))
                sb_k.append(cpool.tile([128, S], bf, name=f"k{p}", tag=f"k{p}"))
                sb_on.append(cpool.tile([128, S], bf, name=f"on{p}", tag=f"on{p}"))
            sb_v = cpool.tile([128, 16, 4, 65], bf, name="v", tag="v")
            nc.vector.memset(sb_v[:, :, :, 64:65], 1.0)

            ats0 = {}
            ats1 = {}

            with tc.tile_pool(name="sc", bufs=1, space="PSUM") as scp:

                def scores_range(p, ats, kb_lo, kb_hi):
                    with nc.named_scope(f"scores_p{p}"), tc.high_priority():
                        for kb in range(kb_lo, kb_hi + 1):
                            q0, q1 = _win(kb)
                            wk_ = q1 - q0
                            j0 = q0 - (kb - 2) * 128
                            sc = scp.tile([128, 2, 1024], f32, name="sc", tag="sc")
                            at = apool.tile([128, 2, WMAX], bf, name="at", tag="at")
                            pieces = [(0, min(wk_, 512))]
                            if wk_ > 512:
                                pieces.append((512, wk_))
                            for a, b in pieces:
                                for hh in range(2):
                                    half = hh * 64
                                    nc.tensor.matmul(
                                        sc[:, hh, a:b],
                                        lhsT=sb_k[p][half:half + 64, kb * 128:(kb + 1) * 128],
                                        rhs=sb_q[p][half:half + 64, q0 + a:q0 + b],
                                        start=True,
                                        stop=True,
                                    )
                                nc.scalar.activation(at[:, :, a:b], sc[:, :, a:b], EXP)
                                nc.vector.tensor_mul(at[:, :, a:b], at[:, :, a:b],
                                                     sb_mask[:, :, j0 + a:j0 + b])
                            ats[kb] = at
                            if debug and p == 0 and kb == 8:
                                nc.sync.dma_start(out=d_dbg_at[:, :], in_=at[:, 0, :])

                # ---- pre-era: pair-0 projections tt-serial (earliest start),
                # interleaved with pair-0 score groups.
                GROUPS = [(0, 1), (2, 5), (6, 9), (10, 15)]
                with tc.tile_pool(name="pre", bufs=2, space="PSUM") as prep:
                    jt = prep.tile([128, 512], f32, name="acc", tag="acc")
                    with nc.named_scope("warmup"):
                        for i in range(48):
                            nc.tensor.matmul(jt[:, 0:256], lhsT=junk[:, 0:128],
                                             rhs=junk, start=True, stop=True)
                    for tt in range(4):
                        sl = slice(tt * 512, (tt + 1) * 512)
                        with nc.named_scope(f"proj_qk0_{tt}"):
                            for wi, (w_sb, dst) in enumerate(((sb_wq, sb_q), (sb_wk, sb_k))):
                                acc = prep.tile([128, 512], f32, name="acc", tag="acc")
                                for dc in range(8):
                                    nc.tensor.matmul(
                                        acc,
                                        lhsT=w_sb[:, dc, 0, :],
                                        rhs=sb_x[dc][:, sl],
                                        start=(dc == 0),
                                        stop=(dc == 7),
                                    )
                                if wi == 0:
                                    nc.scalar.copy(dst[0][:, sl], acc)
                                else:
                                    nc.vector.tensor_copy(dst[0][:, sl], acc)
                        scores_range(0, ats0, *GROUPS[tt])

                # ---- pair-1 projections: dc-outer, weights stationary over
                # 4 query-tile psums.
                with tc.tile_pool(name="pj", bufs=4, space="PSUM") as pj:
                    for scope, w_sb, dst, eng_alt in (
                        ("proj_q1", sb_wq, sb_q[1], 0),
                        ("proj_k1", sb_wk, sb_k[1], 1),
                    ):
                        with nc.named_scope(scope):
                            pss = [pj.tile([128, 512], f32, name="pspj", tag="pj")
                                   for _ in range(4)]
                            for dc in range(8):
                                for tt in range(4):
                                    nc.tensor.matmul(
                                        pss[tt],
                                        lhsT=w_sb[:, dc, 1, :],
                                        rhs=sb_x[dc][:, tt * 512:(tt + 1) * 512],
                                        start=(dc == 0),
                                        stop=(dc == 7),
                                    )
                            for tt in range(4):
                                if (tt + eng_alt) % 2 == 0:
                                    nc.vector.tensor_copy(dst[:, tt * 512:(tt + 1) * 512], pss[tt])
                                else:
                                    nc.scalar.copy(dst[:, tt * 512:(tt + 1) * 512], pss[tt])

                # ---- main era: v-proj chunks, pair-1 score groups, PV
                # quarters, y-proj -- interleaved so PE always has filler
                # work while exp chains drain.
                with (
                    tc.tile_pool(name="ot", bufs=2, space="PSUM") as otp,
                    tc.tile_pool(name="ypsum", bufs=2, space="PSUM") as yps,
                ):
                    def v_chunk(t):
                        with nc.named_scope("proj_v"):
                            ps = yps.tile([128, 256], f32, name="psv", tag="psy")
                            for dc in range(8):
                                nc.tensor.matmul(
                                    ps,
                                    lhsT=sb_x[dc][:, t * 128:(t + 1) * 128],
                                    rhs=sb_wv[:, dc, :],
                                    start=(dc == 0),
                                    stop=(dc == 7),
                                )
                            if t % 2 == 0:
                                nc.scalar.copy(
                                    sb_v[:, t, :, 0:64],
                                    ps.rearrange("p (h d) -> p h d", h=4),
                                )
                            else:
                                nc.vector.tensor_copy(
                                    sb_v[:, t, :, 0:64],
                                    ps.rearrange("p (h d) -> p h d", h=4),
                                )

                    def pv_quarter(p, hh, ats, qt):
                        h = 2 * p + hh
                        half = hh * 64
                        kb_lo = max(0, 4 * qt - 2)
                        kb_hi = min(NKB - 1, 4 * qt + 5)
                        base = qt * 512
                        # kb = 4qt+2's window covers the full quarter: emit it
                        # first with start=True so the whole-bank zero covers
                        # every column before partial pieces accumulate.
                        kb_first = 4 * qt + 2
                        kbs = [kb_first] + [kb for kb in range(kb_lo, kb_hi + 1)
                                            if kb != kb_first]
                        with nc.named_scope(f"pv_h{h}q{qt}"):
                            outq = otp.tile([128, 512], f32, name=f"o{h}{qt}", tag="outp")
                            for i, kb in enumerate(kbs):
                                q0, q1 = _win(kb)
                                a = max(q0, base)
                                b = min(q1, base + 512)
                                nc.tensor.matmul(
                                    outq[0:65, a - base:b - base],
                                    lhsT=sb_v[:, kb, h, :],
                                    rhs=ats[kb][:, hh, a - q0:b - q0],
                                    start=(i == 0),
                                    stop=(i == len(kbs) - 1),
                                )
                            with tc.high_priority():
                                den = spool.tile([1, 512], f32, name="den", tag="den")
                                nc.vector.tensor_copy(den, outq[64:65, :])
                                rec = spool.tile([1, 512], f32, name="rec", tag="rec")
                                nc.vector.reciprocal_approx_fast(rec, den)
                                bc = spool.tile([64, 512], f32, name="bc", tag="bc")
                                nc.gpsimd.partition_broadcast(bc, rec)
                                nc.vector.tensor_mul(
                                    sb_on[p][half:half + 64, base:base + 512],
                                    outq[0:64, :], bc,
                                )

                    def pv_all(qt):
                        for p in range(2):
                            for hh in range(2):
                                pv_quarter(p, hh, ats1 if p else ats0, qt)

                    def yproj_block(qt, dch):
                        with nc.named_scope("proj_y"):
                            for sub in range(2):
                                dc = dch * 2 + sub
                                ps = yps.tile([128, 512], f32, name="psy", tag="psy")
                                for kc in range(2):
                                    nc.tensor.matmul(
                                        ps,
                                        lhsT=sb_wo[:, kc, dc * 128:(dc + 1) * 128],
                                        rhs=sb_on[kc][:, qt * 512:(qt + 1) * 512],
                                        start=(kc == 0),
                                        stop=(kc == 1),
                                    )
                                ysb = ypool.tile([128, 512], bf, name="ysb", tag="ysb")
                                if (qt * 8 + dc) % 2 == 0:
                                    nc.scalar.copy(ysb, ps)
                                else:
                                    nc.vector.tensor_copy(ysb, ps)
                                if qt == 3:
                                    ring = (nc.sync, nc.gpsimd, nc.scalar)[dc % 3]
                                else:
                                    ring = (nc.sync, nc.gpsimd)[dc % 2]
                                ring.dma_start(
                                    out=d_y[:, dc, qt * 512:(qt + 1) * 512],
                                    in_=ysb,
                                )

                    # qt0: v0-5, PV p0, scores g0 (v6-11 as fillers), PV p1, y0
                    for t in range(6):
                        v_chunk(t)
                    pv_quarter(0, 0, ats0, 0)
                    pv_quarter(0, 1, ats0, 0)
                    for i, kb in enumerate(range(0, 6)):
                        scores_range(1, ats1, kb, kb)
                        v_chunk(6 + i)
                    pv_quarter(1, 0, ats1, 0)
                    pv_quarter(1, 1, ats1, 0)

                    # qt1: scores g1 (y0 blocks + v12-13 as fillers), PV, y1...
                    fillers = [
                        lambda: yproj_block(0, 0), lambda: yproj_block(0, 1),
                        lambda: yproj_block(0, 2), lambda: yproj_block(0, 3),
                        lambda: v_chunk(12), lambda: v_chunk(13),
                    ]
                    for i, kb in enumerate(range(6, 10)):
                        scores_range(1, ats1, kb, kb)
                        fillers[i]()
                    fillers[4]()
                    fillers[5]()
                    pv_all(1)

                    fillers = [
                        lambda: yproj_block(1, 0), lambda: yproj_block(1, 1),
                        lambda: yproj_block(1, 2), lambda: yproj_block(1, 3),
                        lambda: v_chunk(14), lambda: v_chunk(15),
                    ]
                    for i, kb in enumerate(range(10, 14)):
                        scores_range(1, ats1, kb, kb)
                        fillers[i]()
                    fillers[4]()
                    fillers[5]()
                    pv_all(2)

                    scores_range(1, ats1, 14, 14)
                    yproj_block(2, 0)
                    yproj_block(2, 1)
                    scores_range(1, ats1, 15, 15)
                    yproj_block(2, 2)
                    yproj_block(2, 3)
                    pv_all(3)

                    if debug:
                        nc.sync.dma_start(out=d_dbg_q[:, :], in_=sb_q[0][:, :])
                        nc.sync.dma_start(out=d_dbg_k[:, :], in_=sb_k[0][:, :])
                        nc.sync.dma_start(
                            out=d_dbg_v[:, :],
                            in_=sb_v.rearrange("p a b c -> p (a b c)"),
                        )
                        nc.sync.dma_start(out=d_dbg_on[:, :], in_=sb_on[0][:, :])

                    for dch in range(4):
                        yproj_block(3, dch)

    nc.compile()
    _cache[key] = nc
    return nc


def kernel(hidden_states, w_q, w_k, w_v, w_o, _debug=False):
    from concourse.bass_utils import run_bass_kernel_spmd

    nc = _build(debug=_debug)
    in_maps = make_in_maps(hidden_states, w_q, w_k, w_v, w_o)
    res = run_bass_kernel_spmd(nc, in_maps, list(range(NCORES)))
    _cache["last_results"] = res

    y = np.zeros((B, S, D), np.float32)
    for c in range(NCORES):
        yT = np.asarray(res.results[c]["yT"], np.float32)  # [128, 8, S]
        y[c // 4] += yT.transpose(1, 0, 2).reshape(D, S).T
    return y


def make_in_maps(hidden_states, w_q, w_k, w_v, w_o):
    mask = _mask_rel()
    scale = np.float32(Dh ** -0.5)

    def chunk_dmajor(w, rows, cols):
        return np.ascontiguousarray(
            w.reshape(rows, 128, cols).transpose(1, 0, 2)
        )

    in_maps = []
    for c in range(NCORES):
        b, hg = c // 4, c % 4
        hsl = slice(hg * 256, (hg + 1) * 256)
        xT = np.asarray(hidden_states[b]).T.astype(bfloat16)  # [D, S]
        wq = chunk_dmajor((np.asarray(w_q[:, hsl]) * scale).astype(bfloat16), 8, 256)
        wk = chunk_dmajor(np.asarray(w_k[:, hsl]).astype(bfloat16), 8, 256)
        in_maps.append({
            "xT": chunk_dmajor(xT, 8, S),
            "wq0": np.ascontiguousarray(wq[:, :, 0:128]),
            "wq1": np.ascontiguousarray(wq[:, :, 128:256]),
            "wk0": np.ascontiguousarray(wk[:, :, 0:128]),
            "wk1": np.ascontiguousarray(wk[:, :, 128:256]),
            "wv": chunk_dmajor(np.asarray(w_v[:, hsl]).astype(bfloat16), 8, 256),
            "wo": chunk_dmajor(np.asarray(w_o[hsl, :]).astype(bfloat16), 2, 1024),
            "maskT": mask,
        })
    return in_maps


# revision 14
# speedup vs baseline: 1.1556x; 1.1556x over previous
"""Dilated attention (segment 64, dilation 4, 16 heads, head_dim 64) on 8 trn2 cores.

Sharding: 2 batches x 4 head-groups (4 heads each) = 8 cores. Each core computes
q/k/v projections for its 4 heads on its batch, block-sparse attention over the
+-2 block (256-token) dilated band, and a partial output projection. Host sums
the 4 head-group partials per batch.

Layout is fully "transposed" on-core to avoid PE transposes:
  xT   [D, S]    (D on partitions, 8 chunks of 128)
  qT/kT [64h, S] per head (head dim on partitions)
  v    [S, 64]   natural (keys on partitions) + ones column -> softmax denoms
  scoresT [k-block 128, q-window <=640] = kT_blk-stationary x qT-window

v3 changes vs v2 (151949 -> 146576 measured):
  - pre-era proj psum double-buffered (bufs=2) so q/k copy overlaps the
    next accumulation chain.
  - pair-1 q/k projections run dc-outer (weights stationary across 4
    query-tile psums) to amortize LDWEIGHTS.
  - exp+mask emitted per 512/128 piece (finer PV wake-up).
  - PV is quarter-major: each (pair,head) accumulates one [65,512]
    quarter psum; its full-cover piece (kb=4qt+2) is emitted first with
    start=True so the whole-bank zero covers every column.
  - normalize: den copy on DVE (NOT ACT -- a PV-dependent copy in the
    exp-heavy ACT FIFO stalls the at-chain and depresses the PE duty
    cycle enough to keep the HAM clock throttled ~20%), and NOT
    high-priority (late scheduling keeps it from blocking mask-muls).
    reciprocal_approx_fast must read SBUF: reading PSUM directly
    returns garbage on HW (sim accepts it).
  - y projection + output DMA per query-quarter, interleaved with
    scores_p1 groups and v-proj chunks as PE gap fillers; tail is only
    the last quarter's yproj + 1MB DMA.
"""

import numpy as np
import ml_dtypes

bfloat16 = ml_dtypes.bfloat16

B, S, D = 2, 2048, 1024
H, Dh = 16, 64
NCORES = 8
NKB = S // 128  # 16 key blocks
WMAX = 640

_cache = {}


def _mask_rel():
    kp = np.arange(128)[:, None]
    j = np.arange(WMAX)[None, :]
    qrel = j - 256
    diff = np.abs(qrel - kp)
    seg = (qrel // 64) == (kp // 64)
    dil = (diff > 0) & (diff % 4 == 0) & (diff <= 256)
    return np.ascontiguousarray((seg | dil).astype(bfloat16))


def _win(kb):
    return max(0, kb - 2) * 128, min(NKB, kb + 3) * 128


def _build(debug=False):
    key = ("nc", debug)
    if key in _cache:
        return _cache[key]
    import concourse.mybir as mybir
    from concourse import bacc
    from concourse.tile import TileContext

    bf = mybir.dt.bfloat16
    f32 = mybir.dt.float32
    EXP = mybir.ActivationFunctionType.Exp

    nc = bacc.Bacc()
    d_x = nc.declare_dram_parameter("xT", [128, 8, S], bf, isOutput=False)
    d_wq0 = nc.declare_dram_parameter("wq0", [128, 8, 128], bf, isOutput=False)
    d_wq1 = nc.declare_dram_parameter("wq1", [128, 8, 128], bf, isOutput=False)
    d_wk0 = nc.declare_dram_parameter("wk0", [128, 8, 128], bf, isOutput=False)
    d_wk1 = nc.declare_dram_parameter("wk1", [128, 8, 128], bf, isOutput=False)
    d_wv = nc.declare_dram_parameter("wv", [128, 8, 256], bf, isOutput=False)
    d_wo = nc.declare_dram_parameter("wo", [128, 2, 1024], bf, isOutput=False)
    d_mask = nc.declare_dram_parameter("maskT", [128, WMAX], bf, isOutput=False)
    d_y = nc.declare_dram_parameter("yT", [128, 8, S], bf, isOutput=True)
    if debug:
        d_dbg_q = nc.declare_dram_parameter("dbg_q", [128, S], bf, isOutput=True)
        d_dbg_k = nc.declare_dram_parameter("dbg_k", [128, S], bf, isOutput=True)
        d_dbg_v = nc.declare_dram_parameter("dbg_v", [128, 16 * 4 * 65], bf, isOutput=True)
        d_dbg_at = nc.declare_dram_parameter("dbg_at", [128, WMAX], bf, isOutput=True)
        d_dbg_on = nc.declare_dram_parameter("dbg_on", [128, S], bf, isOutput=True)

    with TileContext(nc) as tc:
        with (
            tc.tile_pool(name="const", bufs=1) as cpool,
            tc.tile_pool(name="attn", bufs=30) as apool,
            tc.tile_pool(name="ysb", bufs=6) as ypool,
            tc.tile_pool(name="small", bufs=4) as spool,
        ):
            sb_wq = cpool.tile([128, 8, 2, 128], bf, name="wq", tag="wq")
            sb_wk = cpool.tile([128, 8, 2, 128], bf, name="wk", tag="wk")
            sb_wv = cpool.tile([128, 8, 256], bf, name="wv", tag="wv")
            sb_wo = cpool.tile([128, 2, 1024], bf, name="wo", tag="wo")
            sb_mask = cpool.tile([128, WMAX], bf, name="mask", tag="mask")
            sb_xall = cpool.tile([128, 8, S], bf, name="xall", tag="xall")
            sb_x = [sb_xall[:, dc, :] for dc in range(8)]

            junk = cpool.tile([128, 256], bf, name="junk", tag="junk")
            nc.gpsimd.memset(junk, 0.0)

            # ---- input DMA priming: pair-0 weights + x-tt0 first on all
            # 3 rings; later-needed tensors (mask/wv/wq1/wk1/wo) after the
            # x chunks they compete with.
            nc.scalar.dma_start(out=sb_wq[:, :, 0, :], in_=d_wq0[:, :, :])
            nc.sync.dma_start(out=sb_wk[:, :, 0, :], in_=d_wk0[:, :, :])
            nc.gpsimd.dma_start(out=sb_mask, in_=d_mask[:, :])
            for tt in range(4):
                sl = slice(tt * 512, (tt + 1) * 512)
                nc.scalar.dma_start(out=sb_xall[:, 0:3, sl], in_=d_x[:, 0:3, sl])
                nc.sync.dma_start(out=sb_xall[:, 3:6, sl], in_=d_x[:, 3:6, sl])
                nc.gpsimd.dma_start(out=sb_xall[:, 6:8, sl], in_=d_x[:, 6:8, sl])
            nc.scalar.dma_start(out=sb_wq[:, :, 1, :], in_=d_wq1[:, :, :])
            nc.sync.dma_start(out=sb_wk[:, :, 1, :], in_=d_wk1[:, :, :])
            nc.gpsimd.dma_start(out=sb_wv[:, :, :], in_=d_wv[:, :, :])
            nc.sync.dma_start(out=sb_wo[:, 0, :], in_=d_wo[:, 0, :])
            nc.gpsimd.dma_start(out=sb_wo[:, 1, :], in_=d_wo[:, 1, :])

            sb_q = []
            sb_k = []
            sb_on = []
            for p in range(2):
                sb_q.append(cpool.tile([128, S], bf, name=f"q{p}", tag=f"q{p}"))
                sb_k.append(cpool.tile([128, S], bf, name=f"k{p}", tag=f"k{p}"))
                sb_on.append(cpool.tile([128, S], bf, name=f"on{p}", tag=f"on{p}"))
            sb_v = cpool.tile([128, 16, 4, 65], bf, name="v", tag="v")
            nc.vector.memset(sb_v[:, :, :, 64:65], 1.0)

            ats0 = {}
            ats1 = {}

            with tc.tile_pool(name="sc", bufs=1, space="PSUM") as scp:

                def scores_range(p, ats, kb_lo, kb_hi):
                    with nc.named_scope(f"scores_p{p}"), tc.high_priority():
                        for kb in range(kb_lo, kb_hi + 1):
                            q0, q1 = _win(kb)
                            wk_ = q1 - q0
                            j0 = q0 - (kb - 2) * 128
                            sc = scp.tile([128, 2, 1024], f32, name="sc", tag="sc")
                            at = apool.tile([128, 2, WMAX], bf, name="at", tag="at")
                            pieces = [(0, min(wk_, 512))]
                            if wk_ > 512:
                                pieces.append((512, wk_))
                            for a, b in pieces:
                                for hh in range(2):
                                    half = hh * 64
                                    nc.tensor.matmul(
                                        sc[:, hh, a:b],
                                        lhsT=sb_k[p][half:half + 64, kb * 128:(kb + 1) * 128],
                                        rhs=sb_q[p][half:half + 64, q0 + a:q0 + b],
                                        start=True,
                                        stop=True,
                                    )
                                nc.scalar.activation(at[:, :, a:b], sc[:, :, a:b], EXP)
                                mk = sb_mask[:, j0 + a:j0 + b].rearrange(
                                    "p (o w) -> p o w", o=1).broadcast_to([128, 2, b - a])
                                nc.vector.tensor_mul(at[:, :, a:b], at[:, :, a:b], mk)
                            ats[kb] = at
                            if debug and p == 0 and kb == 8:
                                nc.sync.dma_start(out=d_dbg_at[:, :], in_=at[:, 0, :])

                # ---- pre-era: pair-0 projections tt-serial (earliest start),
                # interleaved with pair-0 score groups.
                GROUPS = [(0, 1), (2, 5), (6, 9), (10, 15)]
                with tc.tile_pool(name="pre", bufs=2, space="PSUM") as prep:
                    jt = prep.tile([128, 512], f32, name="acc", tag="acc")
                    with nc.named_scope("warmup"):
                        for i in range(48):
                            nc.tensor.matmul(jt[:, 0:256], lhsT=junk[:, 0:128],
                                             rhs=junk, start=True, stop=True)
                    for tt in range(4):
                        sl = slice(tt * 512, (tt + 1) * 512)
                        with nc.named_scope(f"proj_qk0_{tt}"):
                            for wi, (w_sb, dst) in enumerate(((sb_wq, sb_q), (sb_wk, sb_k))):
                                acc = prep.tile([128, 512], f32, name="acc", tag="acc")
                                for dc in range(8):
                                    nc.tensor.matmul(
                                        acc,
                                        lhsT=w_sb[:, dc, 0, :],
                                        rhs=sb_x[dc][:, sl],
                                        start=(dc == 0),
                                        stop=(dc == 7),
                                    )
                                if wi == 0:
                                    nc.scalar.copy(dst[0][:, sl], acc)
                                else:
                                    nc.vector.tensor_copy(dst[0][:, sl], acc)
                        scores_range(0, ats0, *GROUPS[tt])

                # ---- pair-1 projections: dc-outer, weights stationary over
                # 4 query-tile psums.
                with tc.tile_pool(name="pj", bufs=4, space="PSUM") as pj:
                    for scope, w_sb, dst, eng_alt in (
                        ("proj_q1", sb_wq, sb_q[1], 0),
                        ("proj_k1", sb_wk, sb_k[1], 1),
                    ):
                        with nc.named_scope(scope):
                            pss = [pj.tile([128, 512], f32, name="pspj", tag="pj")
                                   for _ in range(4)]
                            for dc in range(8):
                                for tt in range(4):
                                    nc.tensor.matmul(
                                        pss[tt],
                                        lhsT=w_sb[:, dc, 1, :],
                                        rhs=sb_x[dc][:, tt * 512:(tt + 1) * 512],
                                        start=(dc == 0),
                                        stop=(dc == 7),
                                    )
                            for tt in range(4):
                                if (tt + eng_alt) % 2 == 0:
                                    nc.vector.tensor_copy(dst[:, tt * 512:(tt + 1) * 512], pss[tt])
                                else:
                                    nc.scalar.copy(dst[:, tt * 512:(tt + 1) * 512], pss[tt])

                # ---- main era: v-proj chunks, pair-1 score groups, PV
                # quarters, y-proj -- interleaved so PE always has filler
                # work while exp chains drain.
                with (
                    tc.tile_pool(name="ot", bufs=2, space="PSUM") as otp,
                    tc.tile_pool(name="ypsum", bufs=2, space="PSUM") as yps,
                ):
                    def v_chunk(t):
                        with nc.named_scope("proj_v"):
                            ps = yps.tile([128, 256], f32, name="psv", tag="psy")
                            for dc in range(8):
                                nc.tensor.matmul(
                                    ps,
                                    lhsT=sb_x[dc][:, t * 128:(t + 1) * 128],
                                    rhs=sb_wv[:, dc, :],
                                    start=(dc == 0),
                                    stop=(dc == 7),
                                )
                            if t % 2 == 0:
                                nc.scalar.copy(
                                    sb_v[:, t, :, 0:64],
                                    ps.rearrange("p (h d) -> p h d", h=4),
                                )
                            else:
                                nc.vector.tensor_copy(
                                    sb_v[:, t, :, 0:64],
                                    ps.rearrange("p (h d) -> p h d", h=4),
                                )

                    def pv_quarter(p, hh, ats, qt):
                        h = 2 * p + hh
                        half = hh * 64
                        kb_lo = max(0, 4 * qt - 2)
                        kb_hi = min(NKB - 1, 4 * qt + 5)
                        base = qt * 512
                        # kb = 4qt+2's window covers the full quarter: emit it
                        # first with start=True so the whole-bank zero covers
                        # every column before partial pieces accumulate.
                        kb_first = 4 * qt + 2
                        kbs = [kb_first] + [kb for kb in range(kb_lo, kb_hi + 1)
                                            if kb != kb_first]
                        with nc.named_scope(f"pv_h{h}q{qt}"):
                            outq = otp.tile([128, 512], f32, name=f"o{h}{qt}", tag="outp")
                            for i, kb in enumerate(kbs):
                                q0, q1 = _win(kb)
                                a = max(q0, base)
                                b = min(q1, base + 512)
                                nc.tensor.matmul(
                                    outq[0:65, a - base:b - base],
                                    lhsT=sb_v[:, kb, h, :],
                                    rhs=ats[kb][:, hh, a - q0:b - q0],
                                    start=(i == 0),
                                    stop=(i == len(kbs) - 1),
                                )
                            with tc.high_priority():
                                den = spool.tile([1, 512], f32, name="den", tag="den")
                                nc.vector.tensor_copy(den, outq[64:65, :])
                                rec = spool.tile([1, 512], f32, name="rec", tag="rec")
                                nc.vector.reciprocal_approx_fast(rec, den)
                                bc = spool.tile([64, 512], f32, name="bc", tag="bc")
                                nc.gpsimd.partition_broadcast(bc, rec)
                                nc.vector.tensor_mul(
                                    sb_on[p][half:half + 64, base:base + 512],
                                    outq[0:64, :], bc,
                                )

                    def pv_all(qt):
                        for p in range(2):
                            for hh in range(2):
                                pv_quarter(p, hh, ats1 if p else ats0, qt)

                    def yproj_block(qt, dch):
                        with nc.named_scope("proj_y"):
                            for sub in range(2):
                                dc = dch * 2 + sub
                                ps = yps.tile([128, 512], f32, name="psy", tag="psy")
                                for kc in range(2):
                                    nc.tensor.matmul(
                                        ps,
                                        lhsT=sb_wo[:, kc, dc * 128:(dc + 1) * 128],
                                        rhs=sb_on[kc][:, qt * 512:(qt + 1) * 512],
                                        start=(kc == 0),
                                        stop=(kc == 1),
                                    )
                                ysb = ypool.tile([128, 512], bf, name="ysb", tag="ysb")
                                if (qt * 8 + dc) % 2 == 0:
                                    nc.scalar.copy(ysb, ps)
                                else:
                                    nc.vector.tensor_copy(ysb, ps)
                                if qt == 3:
                                    ring = (nc.sync, nc.gpsimd, nc.scalar)[dc % 3]
                                else:
                                    ring = (nc.sync, nc.gpsimd)[dc % 2]
                                ring.dma_start(
                                    out=d_y[:, dc, qt * 512:(qt + 1) * 512],
                                    in_=ysb,
                                )

                    # qt0: v0-5, PV p0, scores g0 (v6-11 as fillers), PV p1, y0
                    for t in range(6):
                        v_chunk(t)
                    pv_quarter(0, 0, ats0, 0)
                    pv_quarter(0, 1, ats0, 0)
                    for i, kb in enumerate(range(0, 6)):
                        scores_range(1, ats1, kb, kb)
                        v_chunk(6 + i)
                    pv_quarter(1, 0, ats1, 0)
                    pv_quarter(1, 1, ats1, 0)

                    # qt1: scores g1 (y0 blocks + v12-13 as fillers), PV, y1...
                    fillers = [
                        lambda: yproj_block(0, 0), lambda: yproj_block(0, 1),
                        lambda: yproj_block(0, 2), lambda: yproj_block(0, 3),
                        lambda: v_chunk(12), lambda: v_chunk(13),
                    ]
                    for i, kb in enumerate(range(6, 10)):
                        scores_range(1, ats1, kb, kb)
                        fillers[i]()
                    fillers[4]()
                    fillers[5]()
                    pv_all(1)

                    fillers = [
                        lambda: yproj_block(1, 0), lambda: yproj_block(1, 1),
                        lambda: yproj_block(1, 2), lambda: yproj_block(1, 3),
                        lambda: v_chunk(14), lambda: v_chunk(15),
                    ]
                    for i, kb in enumerate(range(10, 14)):
                        scores_range(1, ats1, kb, kb)
                        fillers[i]()
                    fillers[4]()
                    fillers[5]()
                    pv_all(2)

                    scores_range(1, ats1, 14, 14)
                    yproj_block(2, 0)
                    yproj_block(2, 1)
                    scores_range(1, ats1, 15, 15)
                    yproj_block(2, 2)
                    yproj_block(2, 3)
                    pv_all(3)

                    if debug:
                        nc.sync.dma_start(out=d_dbg_q[:, :], in_=sb_q[0][:, :])
                        nc.sync.dma_start(out=d_dbg_k[:, :], in_=sb_k[0][:, :])
                        nc.sync.dma_start(
                            out=d_dbg_v[:, :],
                            in_=sb_v.rearrange("p a b c -> p (a b c)"),
                        )
                        nc.sync.dma_start(out=d_dbg_on[:, :], in_=sb_on[0][:, :])

                    for dch in range(4):
                        yproj_block(3, dch)

    nc.compile()
    _cache[key] = nc
    return nc


def kernel(hidden_states, w_q, w_k, w_v, w_o, _debug=False):
    from concourse.bass_utils import run_bass_kernel_spmd

    nc = _build(debug=_debug)
    in_maps = make_in_maps(hidden_states, w_q, w_k, w_v, w_o)
    res = run_bass_kernel_spmd(nc, in_maps, list(range(NCORES)))
    _cache["last_results"] = res

    y = np.zeros((B, S, D), np.float32)
    for c in range(NCORES):
        yT = np.asarray(res.results[c]["yT"], np.float32)  # [128, 8, S]
        y[c // 4] += yT.transpose(1, 0, 2).reshape(D, S).T
    return y


def make_in_maps(hidden_states, w_q, w_k, w_v, w_o):
    mask = _mask_rel()
    scale = np.float32(Dh ** -0.5)

    def chunk_dmajor(w, rows, cols):
        return np.ascontiguousarray(
            w.reshape(rows, 128, cols).transpose(1, 0, 2)
        )

    in_maps = []
    for c in range(NCORES):
        b, hg = c // 4, c % 4
        hsl = slice(hg * 256, (hg + 1) * 256)
        xT = np.asarray(hidden_states[b]).T.astype(bfloat16)  # [D, S]
        wq = chunk_dmajor((np.asarray(w_q[:, hsl]) * scale).astype(bfloat16), 8, 256)
        wk = chunk_dmajor(np.asarray(w_k[:, hsl]).astype(bfloat16), 8, 256)
        in_maps.append({
            "xT": chunk_dmajor(xT, 8, S),
            "wq0": np.ascontiguousarray(wq[:, :, 0:128]),
            "wq1": np.ascontiguousarray(wq[:, :, 128:256]),
            "wk0": np.ascontiguousarray(wk[:, :, 0:128]),
            "wk1": np.ascontiguousarray(wk[:, :, 128:256]),
            "wv": chunk_dmajor(np.asarray(w_v[:, hsl]).astype(bfloat16), 8, 256),
            "wo": chunk_dmajor(np.asarray(w_o[hsl, :]).astype(bfloat16), 2, 1024),
            "maskT": mask,
        })
    return in_maps


# revision 15
# speedup vs baseline: 1.1755x; 1.0172x over previous
"""Dilated attention (segment 64, dilation 4, 16 heads, head_dim 64) on 8 trn2 cores.

Sharding: 2 batches x 4 head-groups (4 heads each) = 8 cores. Each core computes
q/k/v projections for its 4 heads on its batch, block-sparse attention over the
+-2 block (256-token) dilated band, and a partial output projection. Host sums
the 4 head-group partials per batch.

Layout is fully "transposed" on-core to avoid PE transposes:
  xT   [D, S]    (D on partitions, 8 chunks of 128)
  qT/kT [64h, S] per head (head dim on partitions)
  v    [S, 64]   natural (keys on partitions) + ones column -> softmax denoms
  scoresT [k-block 128, q-window <=640] = kT_blk-stationary x qT-window

v3 changes vs v2 (151949 -> 146576 measured):
  - pre-era proj psum double-buffered (bufs=2) so q/k copy overlaps the
    next accumulation chain.
  - pair-1 q/k projections run dc-outer (weights stationary across 4
    query-tile psums) to amortize LDWEIGHTS.
  - exp+mask emitted per 512/128 piece (finer PV wake-up).
  - PV is quarter-major: each (pair,head) accumulates one [65,512]
    quarter psum; its full-cover piece (kb=4qt+2) is emitted first with
    start=True so the whole-bank zero covers every column.
  - normalize: den copy on DVE (NOT ACT -- a PV-dependent copy in the
    exp-heavy ACT FIFO stalls the at-chain and depresses the PE duty
    cycle enough to keep the HAM clock throttled ~20%), and NOT
    high-priority (late scheduling keeps it from blocking mask-muls).
    reciprocal_approx_fast must read SBUF: reading PSUM directly
    returns garbage on HW (sim accepts it).
  - y projection + output DMA per query-quarter, interleaved with
    scores_p1 groups and v-proj chunks as PE gap fillers; tail is only
    the last quarter's yproj + 1MB DMA.
"""

import numpy as np
import ml_dtypes

bfloat16 = ml_dtypes.bfloat16

B, S, D = 2, 2048, 1024
H, Dh = 16, 64
NCORES = 8
NKB = S // 128  # 16 key blocks
WMAX = 640

_cache = {}


def _mask_rel():
    kp = np.arange(128)[:, None]
    j = np.arange(WMAX)[None, :]
    qrel = j - 256
    diff = np.abs(qrel - kp)
    seg = (qrel // 64) == (kp // 64)
    dil = (diff > 0) & (diff % 4 == 0) & (diff <= 256)
    return np.ascontiguousarray((seg | dil).astype(bfloat16))


def _win(kb):
    return max(0, kb - 2) * 128, min(NKB, kb + 3) * 128


def _build(debug=False):
    key = ("nc", debug)
    if key in _cache:
        return _cache[key]
    import concourse.mybir as mybir
    from concourse import bacc
    from concourse.tile import TileContext

    bf = mybir.dt.bfloat16
    f32 = mybir.dt.float32
    EXP = mybir.ActivationFunctionType.Exp

    nc = bacc.Bacc()
    d_x = nc.declare_dram_parameter("xT", [128, 8, S], bf, isOutput=False)
    d_wq0 = nc.declare_dram_parameter("wq0", [128, 8, 128], bf, isOutput=False)
    d_wq1 = nc.declare_dram_parameter("wq1", [128, 8, 128], bf, isOutput=False)
    d_wk0 = nc.declare_dram_parameter("wk0", [128, 8, 128], bf, isOutput=False)
    d_wk1 = nc.declare_dram_parameter("wk1", [128, 8, 128], bf, isOutput=False)
    d_wv = nc.declare_dram_parameter("wv", [128, 8, 256], bf, isOutput=False)
    d_wo = nc.declare_dram_parameter("wo", [128, 2, 1024], bf, isOutput=False)
    d_mask = nc.declare_dram_parameter("maskT", [128, WMAX], bf, isOutput=False)
    d_y = nc.declare_dram_parameter("yT", [128, 8, S], bf, isOutput=True)
    if debug:
        d_dbg_q = nc.declare_dram_parameter("dbg_q", [128, S], bf, isOutput=True)
        d_dbg_k = nc.declare_dram_parameter("dbg_k", [128, S], bf, isOutput=True)
        d_dbg_v = nc.declare_dram_parameter("dbg_v", [128, 16 * 4 * 65], bf, isOutput=True)
        d_dbg_at = nc.declare_dram_parameter("dbg_at", [128, WMAX], bf, isOutput=True)
        d_dbg_on = nc.declare_dram_parameter("dbg_on", [128, S], bf, isOutput=True)

    with TileContext(nc) as tc:
        with (
            tc.tile_pool(name="const", bufs=1) as cpool,
            tc.tile_pool(name="attn", bufs=30) as apool,
            tc.tile_pool(name="ysb", bufs=6) as ypool,
            tc.tile_pool(name="small", bufs=4) as spool,
        ):
            sb_wq = cpool.tile([128, 8, 2, 128], bf, name="wq", tag="wq")
            sb_wk = cpool.tile([128, 8, 2, 128], bf, name="wk", tag="wk")
            sb_wv = cpool.tile([128, 8, 256], bf, name="wv", tag="wv")
            sb_wo = cpool.tile([128, 2, 1024], bf, name="wo", tag="wo")
            sb_mask = cpool.tile([128, WMAX], bf, name="mask", tag="mask")
            sb_xall = cpool.tile([128, 8, S], bf, name="xall", tag="xall")
            sb_x = [sb_xall[:, dc, :] for dc in range(8)]

            junk = cpool.tile([128, 256], bf, name="junk", tag="junk")
            nc.gpsimd.memset(junk, 0.0)

            # ---- input DMA priming: pair-0 weights + x-tt0 first on all
            # 3 rings; later-needed tensors (mask/wv/wq1/wk1/wo) after the
            # x chunks they compete with.
            nc.scalar.dma_start(out=sb_wq[:, :, 0, :], in_=d_wq0[:, :, :])
            nc.sync.dma_start(out=sb_wk[:, :, 0, :], in_=d_wk0[:, :, :])
            nc.gpsimd.dma_start(out=sb_mask, in_=d_mask[:, :])
            for tt in range(4):
                sl = slice(tt * 512, (tt + 1) * 512)
                nc.scalar.dma_start(out=sb_xall[:, 0:3, sl], in_=d_x[:, 0:3, sl])
                nc.sync.dma_start(out=sb_xall[:, 3:6, sl], in_=d_x[:, 3:6, sl])
                nc.gpsimd.dma_start(out=sb_xall[:, 6:8, sl], in_=d_x[:, 6:8, sl])
            nc.scalar.dma_start(out=sb_wq[:, :, 1, :], in_=d_wq1[:, :, :])
            nc.sync.dma_start(out=sb_wk[:, :, 1, :], in_=d_wk1[:, :, :])
            nc.gpsimd.dma_start(out=sb_wv[:, :, :], in_=d_wv[:, :, :])
            nc.sync.dma_start(out=sb_wo[:, 0, :], in_=d_wo[:, 0, :])
            nc.gpsimd.dma_start(out=sb_wo[:, 1, :], in_=d_wo[:, 1, :])

            sb_q = []
            sb_k = []
            sb_on = []
            for p in range(2):
                sb_q.append(cpool.tile([128, S], bf, name=f"q{p}", tag=f"q{p}"))
                sb_k.append(cpool.tile([128, S], bf, name=f"k{p}", tag=f"k{p}"))
                sb_on.append(cpool.tile([128, S], bf, name=f"on{p}", tag=f"on{p}"))
            sb_v = cpool.tile([128, 16, 4, 65], bf, name="v", tag="v")
            nc.vector.memset(sb_v[:, :, :, 64:65], 1.0)

            ats0 = {}
            ats1 = {}

            with tc.tile_pool(name="sc", bufs=2, space="PSUM") as scp:

                def scores_range(p, ats, kb_lo, kb_hi):
                    with nc.named_scope(f"scores_p{p}"), tc.high_priority():
                        for kb in range(kb_lo, kb_hi + 1):
                            q0, q1 = _win(kb)
                            wk_ = q1 - q0
                            j0 = q0 - (kb - 2) * 128
                            at = apool.tile([128, 2, WMAX], bf, name="at", tag="at")
                            pieces = [(0, min(wk_, 512))]
                            if wk_ > 512:
                                pieces.append((512, wk_))
                            for a, b in pieces:
                                # per-piece psum tile: block n+1's main piece only
                                # waits on block n's main exp, not both pieces
                                sc = scp.tile([128, 2, 512], f32, name="sc", tag="sc")
                                for hh in range(2):
                                    half = hh * 64
                                    nc.tensor.matmul(
                                        sc[:, hh, 0:b - a],
                                        lhsT=sb_k[p][half:half + 64, kb * 128:(kb + 1) * 128],
                                        rhs=sb_q[p][half:half + 64, q0 + a:q0 + b],
                                        start=True,
                                        stop=True,
                                    )
                                nc.scalar.activation(at[:, :, a:b], sc[:, :, 0:b - a], EXP)
                                mk = sb_mask[:, j0 + a:j0 + b].rearrange(
                                    "p (o w) -> p o w", o=1).broadcast_to([128, 2, b - a])
                                nc.vector.tensor_mul(at[:, :, a:b], at[:, :, a:b], mk)
                            ats[kb] = at
                            if debug and p == 0 and kb == 8:
                                nc.sync.dma_start(out=d_dbg_at[:, :], in_=at[:, 0, :])

                # ---- pre-era: pair-0 projections tt-serial (earliest start),
                # interleaved with pair-0 score groups.
                GROUPS = [(0, 1), (2, 5), (6, 9), (10, 15)]
                with tc.tile_pool(name="pre", bufs=2, space="PSUM") as prep:
                    jt = prep.tile([128, 512], f32, name="acc", tag="acc")
                    with nc.named_scope("warmup"):
                        for i in range(64):
                            nc.tensor.matmul(jt[:, 0:256], lhsT=junk[:, 0:128],
                                             rhs=junk, start=True, stop=True)
                    for tt in range(4):
                        sl = slice(tt * 512, (tt + 1) * 512)
                        with nc.named_scope(f"proj_qk0_{tt}"):
                            for wi, (w_sb, dst) in enumerate(((sb_wq, sb_q), (sb_wk, sb_k))):
                                acc = prep.tile([128, 512], f32, name="acc", tag="acc")
                                for dc in range(8):
                                    nc.tensor.matmul(
                                        acc,
                                        lhsT=w_sb[:, dc, 0, :],
                                        rhs=sb_x[dc][:, sl],
                                        start=(dc == 0),
                                        stop=(dc == 7),
                                    )
                                if wi == 0:
                                    nc.scalar.copy(dst[0][:, sl], acc)
                                else:
                                    nc.vector.tensor_copy(dst[0][:, sl], acc)
                        scores_range(0, ats0, *GROUPS[tt])

                # ---- pair-1 projections: dc-outer, weights stationary over
                # 4 query-tile psums.
                with tc.tile_pool(name="pj", bufs=4, space="PSUM") as pj:
                    for scope, w_sb, dst, eng_alt in (
                        ("proj_q1", sb_wq, sb_q[1], 0),
                        ("proj_k1", sb_wk, sb_k[1], 1),
                    ):
                        with nc.named_scope(scope):
                            pss = [pj.tile([128, 512], f32, name="pspj", tag="pj")
                                   for _ in range(4)]
                            for dc in range(8):
                                for tt in range(4):
                                    nc.tensor.matmul(
                                        pss[tt],
                                        lhsT=w_sb[:, dc, 1, :],
                                        rhs=sb_x[dc][:, tt * 512:(tt + 1) * 512],
                                        start=(dc == 0),
                                        stop=(dc == 7),
                                    )
                            for tt in range(4):
                                if (tt + eng_alt) % 2 == 0:
                                    nc.vector.tensor_copy(dst[:, tt * 512:(tt + 1) * 512], pss[tt])
                                else:
                                    nc.scalar.copy(dst[:, tt * 512:(tt + 1) * 512], pss[tt])

                # ---- main era: v-proj chunks, pair-1 score groups, PV
                # quarters, y-proj -- interleaved so PE always has filler
                # work while exp chains drain.
                with (
                    tc.tile_pool(name="ot", bufs=2, space="PSUM") as otp,
                    tc.tile_pool(name="ypsum", bufs=2, space="PSUM") as yps,
                ):
                    def v_chunk(t):
                        with nc.named_scope("proj_v"):
                            ps = yps.tile([128, 256], f32, name="psv", tag="psy")
                            for dc in range(8):
                                nc.tensor.matmul(
                                    ps,
                                    lhsT=sb_x[dc][:, t * 128:(t + 1) * 128],
                                    rhs=sb_wv[:, dc, :],
                                    start=(dc == 0),
                                    stop=(dc == 7),
                                )
                            if t % 2 == 0:
                                nc.scalar.copy(
                                    sb_v[:, t, :, 0:64],
                                    ps.rearrange("p (h d) -> p h d", h=4),
                                )
                            else:
                                nc.vector.tensor_copy(
                                    sb_v[:, t, :, 0:64],
                                    ps.rearrange("p (h d) -> p h d", h=4),
                                )

                    def pv_quarter(p, hh, ats, qt):
                        h = 2 * p + hh
                        half = hh * 64
                        kb_lo = max(0, 4 * qt - 2)
                        kb_hi = min(NKB - 1, 4 * qt + 5)
                        base = qt * 512
                        # kb = 4qt+2's window covers the full quarter: emit it
                        # first with start=True so the whole-bank zero covers
                        # every column before partial pieces accumulate.
                        kb_first = 4 * qt + 2
                        kbs = [kb_first] + [kb for kb in range(kb_lo, kb_hi + 1)
                                            if kb != kb_first]
                        with nc.named_scope(f"pv_h{h}q{qt}"):
                            outq = otp.tile([128, 512], f32, name=f"o{h}{qt}", tag="outp")
                            for i, kb in enumerate(kbs):
                                q0, q1 = _win(kb)
                                a = max(q0, base)
                                b = min(q1, base + 512)
                                nc.tensor.matmul(
                                    outq[0:65, a - base:b - base],
                                    lhsT=sb_v[:, kb, h, :],
                                    rhs=ats[kb][:, hh, a - q0:b - q0],
                                    start=(i == 0),
                                    stop=(i == len(kbs) - 1),
                                )
                            with tc.high_priority():
                                den = spool.tile([1, 512], f32, name="den", tag="den")
                                nc.vector.tensor_copy(den, outq[64:65, :])
                                rec = spool.tile([1, 512], f32, name="rec", tag="rec")
                                nc.vector.reciprocal_approx_fast(rec, den)
                                bc = spool.tile([64, 512], f32, name="bc", tag="bc")
                                nc.gpsimd.partition_broadcast(bc, rec)
                                nc.vector.tensor_mul(
                                    sb_on[p][half:half + 64, base:base + 512],
                                    outq[0:64, :], bc,
                                )

                    def pv_all(qt):
                        for p in range(2):
                            for hh in range(2):
                                pv_quarter(p, hh, ats1 if p else ats0, qt)

                    def yproj_block(qt, dch):
                        with nc.named_scope("proj_y"):
                            for sub in range(2):
                                dc = dch * 2 + sub
                                ps = yps.tile([128, 512], f32, name="psy", tag="psy")
                                for kc in range(2):
                                    nc.tensor.matmul(
                                        ps,
                                        lhsT=sb_wo[:, kc, dc * 128:(dc + 1) * 128],
                                        rhs=sb_on[kc][:, qt * 512:(qt + 1) * 512],
                                        start=(kc == 0),
                                        stop=(kc == 1),
                                    )
                                ysb = ypool.tile([128, 512], bf, name="ysb", tag="ysb")
                                if (qt * 8 + dc) % 2 == 0:
                                    nc.scalar.copy(ysb, ps)
                                else:
                                    nc.vector.tensor_copy(ysb, ps)
                                if qt == 3:
                                    ring = (nc.sync, nc.gpsimd, nc.scalar)[dc % 3]
                                else:
                                    ring = (nc.sync, nc.gpsimd)[dc % 2]
                                ring.dma_start(
                                    out=d_y[:, dc, qt * 512:(qt + 1) * 512],
                                    in_=ysb,
                                )

                    # qt0: v0-5, PV p0, scores g0 (v6-11 as fillers), PV p1, y0
                    for t in range(6):
                        v_chunk(t)
                    pv_quarter(0, 0, ats0, 0)
                    pv_quarter(0, 1, ats0, 0)
                    for i, kb in enumerate(range(0, 6)):
                        scores_range(1, ats1, kb, kb)
                        v_chunk(6 + i)
                    pv_quarter(1, 0, ats1, 0)
                    pv_quarter(1, 1, ats1, 0)

                    # qt1: scores g1 (y0 blocks + v12-13 as fillers), PV, y1...
                    fillers = [
                        lambda: yproj_block(0, 0), lambda: yproj_block(0, 1),
                        lambda: yproj_block(0, 2), lambda: yproj_block(0, 3),
                        lambda: v_chunk(12), lambda: v_chunk(13),
                    ]
                    for i, kb in enumerate(range(6, 10)):
                        scores_range(1, ats1, kb, kb)
                        fillers[i]()
                    fillers[4]()
                    fillers[5]()
                    pv_all(1)

                    fillers = [
                        lambda: yproj_block(1, 0), lambda: yproj_block(1, 1),
                        lambda: yproj_block(1, 2), lambda: yproj_block(1, 3),
                        lambda: v_chunk(14), lambda: v_chunk(15),
                    ]
                    for i, kb in enumerate(range(10, 14)):
                        scores_range(1, ats1, kb, kb)
                        fillers[i]()
                    fillers[4]()
                    fillers[5]()
                    pv_all(2)

                    scores_range(1, ats1, 14, 14)
                    yproj_block(2, 0)
                    yproj_block(2, 1)
                    scores_range(1, ats1, 15, 15)
                    yproj_block(2, 2)
                    yproj_block(2, 3)
                    pv_all(3)

                    if debug:
                        nc.sync.dma_start(out=d_dbg_q[:, :], in_=sb_q[0][:, :])
                        nc.sync.dma_start(out=d_dbg_k[:, :], in_=sb_k[0][:, :])
                        nc.sync.dma_start(
                            out=d_dbg_v[:, :],
                            in_=sb_v.rearrange("p a b c -> p (a b c)"),
                        )
                        nc.sync.dma_start(out=d_dbg_on[:, :], in_=sb_on[0][:, :])

                    for dch in range(4):
                        yproj_block(3, dch)

    nc.compile()
    _cache[key] = nc
    return nc


def kernel(hidden_states, w_q, w_k, w_v, w_o, _debug=False):
    from concourse.bass_utils import run_bass_kernel_spmd

    nc = _build(debug=_debug)
    in_maps = make_in_maps(hidden_states, w_q, w_k, w_v, w_o)
    res = run_bass_kernel_spmd(nc, in_maps, list(range(NCORES)))
    _cache["last_results"] = res

    y = np.zeros((B, S, D), np.float32)
    for c in range(NCORES):
        yT = np.asarray(res.results[c]["yT"], np.float32)  # [128, 8, S]
        y[c // 4] += yT.transpose(1, 0, 2).reshape(D, S).T
    return y


def make_in_maps(hidden_states, w_q, w_k, w_v, w_o):
    mask = _mask_rel()
    scale = np.float32(Dh ** -0.5)

    def chunk_dmajor(w, rows, cols):
        return np.ascontiguousarray(
            w.reshape(rows, 128, cols).transpose(1, 0, 2)
        )

    in_maps = []
    for c in range(NCORES):
        b, hg = c // 4, c % 4
        hsl = slice(hg * 256, (hg + 1) * 256)
        xT = np.asarray(hidden_states[b]).T.astype(bfloat16)  # [D, S]
        wq = chunk_dmajor((np.asarray(w_q[:, hsl]) * scale).astype(bfloat16), 8, 256)
        wk = chunk_dmajor(np.asarray(w_k[:, hsl]).astype(bfloat16), 8, 256)
        in_maps.append({
            "xT": chunk_dmajor(xT, 8, S),
            "wq0": np.ascontiguousarray(wq[:, :, 0:128]),
            "wq1": np.ascontiguousarray(wq[:, :, 128:256]),
            "wk0": np.ascontiguousarray(wk[:, :, 0:128]),
            "wk1": np.ascontiguousarray(wk[:, :, 128:256]),
            "wv": chunk_dmajor(np.asarray(w_v[:, hsl]).astype(bfloat16), 8, 256),
            "wo": chunk_dmajor(np.asarray(w_o[hsl, :]).astype(bfloat16), 2, 1024),
            "maskT": mask,
        })
    return in_maps
